# revision 1
# baseline (speedup 1.0000x reference)
"""Multi-head attention (16 heads, S=2048, d_model=1024, d_head=64) on 8 TRN2
NeuronCores, tensor-parallel over heads (2 heads per core).

Numerics: Q/K/V and the QKV projection weights ship as fp16 (e5m10, ~4.9e-4
element precision, half the DMA bytes); projection matmuls run fp16 with fp32
PSUM accumulation. Everything downstream runs float32r (fp32 storage, rne-11
matmul-input rounding, full bf16-rate on the PE). Softmax: zT is computed
transposed (sk on partitions, sq free), exp on ScalarE with the 1/sqrt(d_k)
scale fused, denominator from a ones-column appended to Vp so the P@V matmul
accumulates it for free.

Schedule (PE runs a static FIFO, so emission order is the schedule):
  phase A: per 512-chunk c, stream K,Q projections, then z+exp for every
  feasible (head, sq-group, sk-chunk), then V projection + PE-transposes,
  then the P@V accumulations. Attention for the first two sq chunks rides
  inside the DMA window this way.
  post A: remaining sq chunks run their attention with the previous chunks'
  normalize/output-projection units software-pipelined into the stream.

Host side only transposes/casts/packs inputs and sums the 8 partial outputs.
"""

import os

import numpy as np

import concourse.bass as bass
import concourse.tile as tile
from concourse import bacc, mybir
from concourse.bass_utils import run_bass_kernel_spmd

HEADS, D_K, D_V, D_X, D_M, S = 16, 64, 64, 1024, 1024, 2048
NCORES = 8
HPC = HEADS // NCORES          # heads per core
HD = HPC * D_K                 # 128: stacked head dim per core
SQW = 512                      # sq chunk width (PSUM bank = 512 fp32)
NSQ = S // SQW                 # 4
SKW = 128                      # sk chunk width (partition dim)
NSK = S // SKW                 # 16
NXC = D_X // 128               # 8 contraction chunks for projections
NJ = SQW // SKW                # 4 sk 128-chunks per 512 chunk

F32 = mybir.dt.float32
F32R = mybir.dt.float32r
F16 = mybir.dt.float16
EXP = mybir.ActivationFunctionType.Exp

LAST_EXEC_NS = None
_NC_CACHE = None


def _emit(tc, nc, aps):
    from contextlib import ExitStack

    qt, kt, vt, wq, wk, wv, wot, onescol, out = (
        aps["qt"], aps["kt"], aps["vt"], aps["wq"], aps["wk"], aps["wv"],
        aps["wot"], aps["onescol"], aps["out"],
    )

    with ExitStack() as ctx:
        wpool = ctx.enter_context(tc.tile_pool(name="weights", bufs=1))
        proj = ctx.enter_context(tc.tile_pool(name="proj", bufs=1))
        inp = ctx.enter_context(tc.tile_pool(name="inp", bufs=5))
        etp = ctx.enter_context(tc.tile_pool(name="et", bufs=42))
        outp = ctx.enter_context(tc.tile_pool(name="outs", bufs=4))
        smalls = ctx.enter_context(tc.tile_pool(name="smalls", bufs=2))
        ps_proj = ctx.enter_context(tc.tile_pool(name="ps_proj", bufs=2, space="PSUM"))
        ps_z = ctx.enter_context(tc.tile_pool(name="ps_z", bufs=2, space="PSUM"))
        ps_o = ctx.enter_context(tc.tile_pool(name="ps_o", bufs=4, space="PSUM"))

        # ---- persistent SBUF tensors ----
        wq_sb = wpool.tile([128, D_X], F16, tag="wq")     # (xc p) stacked chunks
        wk_sb = wpool.tile([128, D_X], F16, tag="wk")
        wv_sb = wpool.tile([128, D_X], F16, tag="wv")
        wot_sb = wpool.tile([HD, D_M], F32R, tag="wot")
        qpt_sb = proj.tile([HD, S], F32R, tag="qpt")
        kpt_sb = proj.tile([HD, S], F32R, tag="kpt")
        # VpAug: per (h, skc) a (128 sk, 65) block: cols 0-63 = Vp, col 64 = 1
        vpa_sb = proj.tile([128, HPC * NSK * 65], F32R, tag="vpa")
        headst_sb = proj.tile([HD, S], F32R, tag="headst")

        # weight DMAs are interleaved with the first chunk loads so the
        # K/Q data the first attention steps need is in flight ASAP.
        def load_w(w_dram, w_sb):
            nc.sync.dma_start(
                w_sb[:].rearrange("p (c f) -> p c f", f=128),
                w_dram.rearrange("(c p) f -> p c f", p=128),
            )

        def load_aux():
            onescol_r = onescol.bitcast(F32R)
            # ones column of VpAug (col 64 of each 65-wide block)
            nc.sync.dma_start(
                vpa_sb[:].rearrange("p (c f) -> p c f", f=65)[:, :, 64:65],
                onescol_r[:, 0:HPC * NSK].rearrange("p (c o) -> p c o", o=1),
            )
            nc.sync.dma_start(wot_sb[:], wot.bitcast(F32R))

        def load_chunk(tt_dram, c, name):
            """One DMA: all 8 xc strips of a 512-wide chunk -> (128, 8, 512)."""
            t = inp.tile([128, NXC, SQW], F16, tag="inp", name=name)
            nc.sync.dma_start(
                t[:],
                tt_dram.rearrange("(xc p) s -> p xc s", p=128)[
                    :, :, c * SQW:(c + 1) * SQW
                ],
            )
            return t

        def project(t, w_sb, dst_sb, c):
            """Compute dst_sb[:, c*512:(c+1)*512] = W.T @ X.T chunk (fp16)."""
            ps = ps_proj.tile([128, SQW], F32, tag="ps_proj")
            for xc in range(NXC):
                nc.tensor.matmul(
                    ps[:],
                    w_sb[:, xc * 128:(xc + 1) * 128],
                    t[:, xc, :],
                    start=(xc == 0),
                    stop=(xc == NXC - 1),
                )
            nc.vector.tensor_copy(dst_sb[:, c * SQW:(c + 1) * SQW], ps[:])

        def project_v(t, c):
            """VpAug sk-chunks for 512-chunk c: Vp = VT_chunk.T @ Wv directly
            in (sk, hd) layout (fp16, N=128 runs at full rate)."""
            for j in range(NJ):
                skc = c * NJ + j
                ps = ps_proj.tile([128, HD], F32, tag="ps_proj", name=f"vp_{skc}")
                for xc in range(NXC):
                    nc.tensor.matmul(
                        ps[:],
                        t[:, xc, j * SKW:(j + 1) * SKW],
                        wv_sb[:, xc * 128:(xc + 1) * 128],
                        start=(xc == 0),
                        stop=(xc == NXC - 1),
                    )
                for h in range(HPC):
                    base = (h * NSK + skc) * 65
                    nc.vector.tensor_copy(
                        vpa_sb[:, base:base + 64],
                        ps[:, h * 64:(h + 1) * 64],
                    )

        def z_exp(h, sqc, skc):
            """z matmul + exp; returns the ET tile for the AV step."""
            z_ps = ps_z.tile([128, SQW], F32, tag="ps_z")
            nc.tensor.matmul(
                z_ps[:],
                kpt_sb[h * 64:(h + 1) * 64, skc * SKW:(skc + 1) * SKW],
                qpt_sb[h * 64:(h + 1) * 64, sqc * SQW:(sqc + 1) * SQW],
                start=True,
                stop=True,
            )
            et = etp.tile([128, SQW], F32R, tag="et")
            nc.scalar.activation(et[:], z_ps[:], EXP, scale=1.0 / 8.0)
            return et

        def av(h, skc, et, o_ps):
            base = (h * NSK + skc) * 65
            nc.tensor.matmul(
                o_ps[:],
                vpa_sb[:, base:base + 65],
                et[:],
                start=(skc == 0),
                stop=(skc == NSK - 1),
            )

        def normalize(h, sqc, o_ps, copy_engine="vector"):
            """heads_h[:, sqc chunk] = o[0:64] / o[64]. The reciprocal row is
            replicated across partitions on the idle GpSimd engine."""
            rec = smalls.tile([1, SQW], F32, tag="rec")
            nc.vector.reciprocal(rec[:], o_ps[64:65, :])
            rec64 = smalls.tile([64, SQW], F32, tag="rec64")
            nc.gpsimd.partition_broadcast(rec64[:], rec[:])
            nc.vector.tensor_mul(
                headst_sb[h * 64:(h + 1) * 64, sqc * SQW:(sqc + 1) * SQW],
                o_ps[0:64, :],
                rec64[:],
            )

        def outproj_unit(sqt, copy_engine="vector"):
            """One out[128 x 1024] row-tile: 2 matmuls, 2 copies, 1 DMA."""
            ot = outp.tile([128, D_M], F32, tag="ot")
            for dmc in range(D_M // SQW):
                op = ps_proj.tile([128, SQW], F32, tag="ps_proj")
                nc.tensor.matmul(
                    op[:],
                    headst_sb[:, sqt * 128:(sqt + 1) * 128],
                    wot_sb[:, dmc * SQW:(dmc + 1) * SQW],
                    start=True,
                    stop=True,
                )
                dst = ot[:, dmc * SQW:(dmc + 1) * SQW]
                if copy_engine == "scalar":
                    nc.scalar.copy(dst, op[:])
                else:
                    nc.vector.tensor_copy(dst, op[:])
            nc.sync.dma_start(out[sqt * 128:(sqt + 1) * 128, :], ot[:])

        def outproj_units(sqc):
            return [
                (lambda copy_engine="vector", sqt=sqc * (SQW // 128) + j:
                 outproj_unit(sqt, copy_engine))
                for j in range(SQW // 128)
            ]

        # ---- phase A: stream projections. Attention z/exp is emitted for sq
        # chunks 0..2 as soon as their K/Q chunks land (PE FIFO never blocks
        # on a pending AV); AVs accumulate for chunks 0,1 (4 PSUM banks),
        # while chunk 2's ET tiles are stored in SBUF for post-A AVs. ----
        NG0 = 2       # groups accumulated during phase A
        NGE = 3       # groups whose z/exp runs during phase A
        oa = {}
        ets2 = {}     # (h, skc) -> stored ET tile for sq chunk 2
        for c in range(NSQ):
            if c == 0:
                tk = load_chunk(kt, c, f"kc_{c}")
                load_w(wk, wk_sb)
                tq = load_chunk(qt, c, f"qc_{c}")
                load_w(wq, wq_sb)
                tv = load_chunk(vt, c, f"vc_{c}")
                load_w(wv, wv_sb)
                load_aux()
            else:
                tk = load_chunk(kt, c, f"kc_{c}")
                tq = load_chunk(qt, c, f"qc_{c}")
                tv = load_chunk(vt, c, f"vc_{c}")
            project(tk, wk_sb, kpt_sb, c)
            project(tq, wq_sb, qpt_sb, c)
            steps = []     # (h, g, skc) feasible this round
            for g in range(NGE):
                if g > c:
                    continue
                lo = 0 if g == c else c * NJ
                for skc in range(lo, (c + 1) * NJ):
                    for h in range(HPC):
                        steps.append((h, g, skc))
            ets = [z_exp(h, g, skc) for (h, g, skc) in steps]
            project_v(tv, c)
            for (h, g, skc), et in zip(steps, ets):
                if g >= NG0:
                    ets2[(h, skc)] = et
                    continue
                if (h, g) not in oa:
                    oa[(h, g)] = ps_o.tile([65, SQW], F32, tag="ps_o",
                                           name=f"oa_{h}_{g}")
                av(h, skc, et, oa[(h, g)])
            if c == NSQ - 1:
                # chunks 0/1 are complete: normalize them now, freeing their
                # PSUM banks so chunk 2's stored AVs run before post-A
                for g in range(NG0):
                    for h in range(HPC):
                        normalize(h, g, oa[(h, g)])
                ob2 = {h: ps_o.tile([65, SQW], F32, tag="ps_o",
                                    name=f"ob2_{h}")
                       for h in range(HPC)}
                for skc in range(NSK):
                    for h in range(HPC):
                        av(h, skc, ets2.pop((h, skc)), ob2[h])
                for h in range(HPC):
                    normalize(h, 2, ob2[h])

        # ---- post A: only chunk 3's attention remains; chunks 0/1/2's
        # outproj units interleave into its stream as PE fillers ----
        fillers = outproj_units(0) + outproj_units(1) + outproj_units(2)
        ob3 = {h: ps_o.tile([65, SQW], F32, tag="ps_o", name=f"ob3_{h}")
               for h in range(HPC)}
        fi = 0
        pend = []   # chunk-3 AVs trail its z/exp stream by one skc
        for skc in range(NSK):
            nxt = [(h, skc, z_exp(h, 3, skc)) for h in range(HPC)]
            for h, pskc, et in pend:
                av(h, pskc, et, ob3[h])
            pend = nxt
            if fi < len(fillers):
                fillers[fi]()
                fi += 1
        for h, pskc, et in pend:
            av(h, pskc, et, ob3[h])
        while fi < len(fillers):
            fillers[fi]()
            fi += 1
        for h in range(HPC):
            normalize(h, 3, ob3[h], copy_engine="scalar")
        for i, f in enumerate(outproj_units(3)):
            f(copy_engine="scalar" if i % 2 else "vector")


def _build_nc():
    nc = bacc.Bacc("TRN2", target_bir_lowering=False, debug=False,
                   num_devices=NCORES)
    aps = {
        "qt": nc.dram_tensor("qt", [D_X, S], F16, kind="ExternalInput").ap(),
        "kt": nc.dram_tensor("kt", [D_X, S], F16, kind="ExternalInput").ap(),
        "vt": nc.dram_tensor("vt", [D_X, S], F16, kind="ExternalInput").ap(),
        "wq": nc.dram_tensor("wq", [D_X, HD], F16, kind="ExternalInput").ap(),
        "wk": nc.dram_tensor("wk", [D_X, HD], F16, kind="ExternalInput").ap(),
        "wv": nc.dram_tensor("wv", [D_X, HD], F16, kind="ExternalInput").ap(),
        "wot": nc.dram_tensor("wot", [HD, D_M], F32, kind="ExternalInput").ap(),
        "onescol": nc.dram_tensor("onescol", [128, 64], F32, kind="ExternalInput").ap(),
        "out": nc.dram_tensor("out", [S, D_M], F32, kind="ExternalOutput").ap(),
    }
    with tile.TileContext(nc) as tc:
        with nc.allow_low_precision(reason="f32r/fp16 matmul pipeline"):
            _emit(tc, nc, aps)
    nc.compile()
    return nc


def kernel(**inputs):
    global LAST_EXEC_NS, _NC_CACHE
    Q = np.asarray(inputs["Q"], dtype=np.float32)
    K = np.asarray(inputs["K"], dtype=np.float32)
    V = np.asarray(inputs["V"], dtype=np.float32)
    W_q = np.asarray(inputs["W_q"], dtype=np.float32)
    W_k = np.asarray(inputs["W_k"], dtype=np.float32)
    W_v = np.asarray(inputs["W_v"], dtype=np.float32)
    W_o = np.asarray(inputs["W_o"], dtype=np.float32)

    QT = np.ascontiguousarray(Q.T.astype(np.float16))
    KT = np.ascontiguousarray(K.T.astype(np.float16))
    VT = np.ascontiguousarray(V.T.astype(np.float16))
    in_maps = []
    for c in range(NCORES):
        h0 = HPC * c
        in_maps.append({
            "qt": QT, "kt": KT, "vt": VT,
            "wq": np.ascontiguousarray(np.concatenate(
                [W_q[h0 + i] for i in range(HPC)], axis=1).astype(np.float16)),
            "wk": np.ascontiguousarray(np.concatenate(
                [W_k[h0 + i] for i in range(HPC)], axis=1).astype(np.float16)),
            "wv": np.ascontiguousarray(np.concatenate(
                [W_v[h0 + i] for i in range(HPC)], axis=1).astype(np.float16)),
            "wot": np.ascontiguousarray(W_o[:, c * HD:(c + 1) * HD].T),
            "onescol": np.ones((128, 64), np.float32),
        })

    if _NC_CACHE is None:
        _NC_CACHE = _build_nc()
    nc = _NC_CACHE

    trace = bool(os.environ.get("MHA_TRACE"))
    res = None
    if trace:
        try:
            res = run_bass_kernel_spmd(nc, in_maps, list(range(NCORES)),
                                       trace=True)
        except Exception as e:  # profiling infra unavailable -> run untraced
            print(f"[kernel] traced run failed ({e!r}); falling back")
            res = None
    if res is None:
        res = run_bass_kernel_spmd(nc, in_maps, list(range(NCORES)))

    LAST_EXEC_NS = getattr(res, "exec_time_ns", None)

    out = np.zeros((S, D_M), np.float32)
    for r in res.results:
        out += r["out"]
    return out



# revision 7
# speedup vs baseline: 1.1283x; 1.1283x over previous
"""Multi-head attention (16 heads, S=2048, d_model=1024, d_head=64) on 8 TRN2
NeuronCores, tensor-parallel over heads (2 heads per core).

Restructured from the 120us baseline around the TimelineSim cost model, where
matmul cost = output-free-size rows (K and M are free):

  * AV matmuls run transposed: out[sq=128, dv+1=65] with the exp tile as the
    stationary operand, 65 rows per accumulation step instead of 512 -> PE
    drops from 196k to 166k rows (~82us -> ~69us busy).
  * softmax normalize becomes a per-partition tensor_scalar (denominator is
    column 64 of the accumulator) -- no gpsimd broadcast.
  * z PSUM tiles are [128, 1024] (2 banks); exp runs 64x instead of 128x,
    halving the fixed per-instruction ACT overhead (~81us -> ~66us busy).
  * everything 2-byte: fp16 in/out of every matmul, exp computes
    exp(z/8 - 4) so the scores fit fp16 (max z/8 ~ 11.9), the bias cancels
    in the normalize. Output ships fp16 (half the out DMA), host sums in f32.
  * heads come out of AV as [sq, dv]; a 16x[128,128] PE transpose (+DVE copy)
    restores [hd, sq] for the output projection.

Schedule: inputs stream K0,Q0,K1,Q1,V0,K2,Q2,V1,K3,Q3,V2,V3 so the exp
stream (the ~66us ACT critical path, the pacer) starts by ~7us and never
starves. z/exp tiles are emitted greedily (group-ascending); group 0's AV
rides inline behind the V projections; groups 1-3 accumulate at the end of
the stream in group order, each followed by its normalize/transpose/outproj/
DMA chain so only group 3's chain is a tail. PSUM: 2x[128,1024] z +
2x[128,260] AV accumulators + 2x[128,512] proj/outproj = exactly 8 banks.
"""

import os

import numpy as np

import concourse.bass as bass
import concourse.tile as tile
from concourse import bacc, mybir
from concourse.bass_utils import run_bass_kernel_spmd

HEADS, D_K, D_V, D_X, D_M, S = 16, 64, 64, 1024, 1024, 2048
NCORES = 8
HPC = HEADS // NCORES          # heads per core
HD = HPC * D_K                 # 128: stacked head dim per core
SQW = 512                      # sq group width
NSQ = S // SQW                 # 4 groups
SKW = 128                      # sk chunk width (partition dim)
NSK = S // SKW                 # 16
NXC = D_X // 128               # 8 contraction chunks for projections
NJJ = NSK // 2                 # 8 skc-pairs (one [128,1024] z tile each)

F32 = mybir.dt.float32
F16 = mybir.dt.float16
EXP = mybir.ActivationFunctionType.Exp

LAST_EXEC_NS = None
_NC_CACHE = None


def _emit(tc, nc, aps):
    from contextlib import ExitStack

    qt, kt, vt, wq, wk, wv, wot, ident, out = (
        aps["qt"], aps["kt"], aps["vt"], aps["wq"], aps["wk"], aps["wv"],
        aps["wot"], aps["ident"], aps["out"],
    )

    with ExitStack() as ctx:
        wpool = ctx.enter_context(tc.tile_pool(name="weights", bufs=1))
        proj = ctx.enter_context(tc.tile_pool(name="proj", bufs=1))
        inp = ctx.enter_context(tc.tile_pool(name="inp", bufs=6))
        etp = ctx.enter_context(tc.tile_pool(name="et", bufs=52))
        hsqp = ctx.enter_context(tc.tile_pool(name="hsq", bufs=8))
        outp = ctx.enter_context(tc.tile_pool(name="outs", bufs=4))
        smalls = ctx.enter_context(tc.tile_pool(name="smalls", bufs=4))
        ps_z = ctx.enter_context(tc.tile_pool(name="ps_z", bufs=2, space="PSUM"))
        ps_av = ctx.enter_context(tc.tile_pool(name="ps_av", bufs=2, space="PSUM"))
        ps_pr = ctx.enter_context(tc.tile_pool(name="ps_pr", bufs=2, space="PSUM"))

        # ---- persistent SBUF tensors ----
        wq_sb = wpool.tile([128, D_X], F16, tag="wq")     # (xc p) stacked chunks
        wk_sb = wpool.tile([128, D_X], F16, tag="wk")
        wv_sb = wpool.tile([128, D_X], F16, tag="wv")
        wot_sb = wpool.tile([HD, D_M], F16, tag="wot")
        ident_sb = wpool.tile([128, 128], F16, tag="ident")
        qpt_sb = proj.tile([HD, S], F16, tag="qpt")
        kpt_sb = proj.tile([HD, S], F16, tag="kpt")
        # VpAug: per (h, skc) a (128 sk, 65) block: cols 0-63 = Vp, col 64 = 1
        vpa_sb = proj.tile([128, HPC * NSK * 65], F16, tag="vpa")
        headst_sb = proj.tile([HD, S], F16, tag="headst")

        def load_w(w_dram, w_sb):
            nc.sync.dma_start(
                w_sb[:].rearrange("p (c f) -> p c f", f=128),
                w_dram.rearrange("(c p) f -> p c f", p=128),
            )

        def load_chunk(tt_dram, c, name):
            """One DMA: all 8 xc strips of a 512-wide chunk -> (128, 8, 512)."""
            t = inp.tile([128, NXC, SQW], F16, tag="inp", name=name)
            nc.sync.dma_start(
                t[:],
                tt_dram.rearrange("(xc p) s -> p xc s", p=128)[
                    :, :, c * SQW:(c + 1) * SQW
                ],
            )
            return t

        def project(t, w_sb, dst_sb, c, name):
            """dst_sb[:, c*512:(c+1)*512] = W.T @ X.T chunk (fp16)."""
            ps = ps_pr.tile([128, SQW], F32, tag="pr", name=name)
            for xc in range(NXC):
                nc.tensor.matmul(
                    ps[:],
                    w_sb[:, xc * 128:(xc + 1) * 128],
                    t[:, xc, :],
                    start=(xc == 0),
                    stop=(xc == NXC - 1),
                )
            nc.vector.tensor_copy(dst_sb[:, c * SQW:(c + 1) * SQW], ps[:])

        def project_v(t, c):
            """VpAug sk-chunks for 512-chunk c: Vp = VT_chunk.T @ Wv directly
            in (sk, hd) layout."""
            for j in range(SQW // SKW):
                skc = c * (SQW // SKW) + j
                ps = ps_pr.tile([128, HD], F32, tag="pr", name=f"vp_{skc}",
                                padded_shape=[128, SQW])
                for xc in range(NXC):
                    nc.tensor.matmul(
                        ps[:],
                        t[:, xc, j * SKW:(j + 1) * SKW],
                        wv_sb[:, xc * 128:(xc + 1) * 128],
                        start=(xc == 0),
                        stop=(xc == NXC - 1),
                    )
                for h in range(HPC):
                    base = (h * NSK + skc) * 65
                    nc.vector.tensor_copy(
                        vpa_sb[:, base:base + 64],
                        ps[:, h * 64:(h + 1) * 64],
                    )

        ets = {}  # (h, g, jj) -> ET tile awaiting its AV matmuls

        def z_exp(g, jj):
            """Per head: one [128,1024] z tile (skc pair 2jj,2jj+1) + exp.

            exp(z/8 - 4): the -4 bias keeps the scores in fp16 range
            (max z/8 ~ 11.9 -> e^7.9 = 2.7e3) and cancels in the normalize.
            """
            for h in range(HPC):
                z_ps = ps_z.tile([128, 2 * SQW], F32, tag="z",
                                 name=f"z_{h}_{g}_{jj}")
                for half in range(2):
                    skc = 2 * jj + half
                    nc.tensor.matmul(
                        z_ps[:, half * SQW:(half + 1) * SQW],
                        kpt_sb[h * 64:(h + 1) * 64, skc * SKW:(skc + 1) * SKW],
                        qpt_sb[h * 64:(h + 1) * 64, g * SQW:(g + 1) * SQW],
                        start=True,
                        stop=True,
                    )
                et = etp.tile([128, 2 * SQW], F16, tag="et",
                              name=f"et_{h}_{g}_{jj}")
                nc.scalar.activation(et[:], z_ps[:], EXP,
                                     scale=1.0 / 8.0, bias=bias_sb[:])
                ets[(h, g, jj)] = et

        # PSUM accumulation groups must be contiguous per bank (interleaved
        # start/stop groups at different offsets in one bank corrupt the
        # result), so AV runs as per-(h, sq-128-subtile) bursts of 16
        # back-to-back matmuls, one bank each, after the group's exps.
        def av_run(g, h, m):
            acc = ps_av.tile([128, 65], F32, tag="av", name=f"av_{g}_{h}_{m}")
            for jj in range(NJJ):
                et = ets[(h, g, jj)]
                for half in range(2):
                    skc = 2 * jj + half
                    vb = (h * NSK + skc) * 65
                    nc.tensor.matmul(
                        acc[:],
                        et[:, half * SQW + m * 128:half * SQW + (m + 1) * 128],
                        vpa_sb[:, vb:vb + 65],
                        start=(skc == 0),
                        stop=(skc == NSK - 1),
                    )
            return acc

        def unit(g, m, copy_split=False):
            """One sq-128 tile end to end: 2 AV runs -> normalize ->
            transpose -> output projection -> DMA."""
            t = g * 4 + m
            accs = [av_run(g, h, m) for h in range(HPC)]
            hsq = hsqp.tile([128, HD], F16, tag="hsq", name=f"hsq_{t}")
            for h in range(HPC):
                rec = smalls.tile([128, 1], F32, tag="rec", name=f"rec_{t}_{h}")
                nc.vector.reciprocal(rec[:], accs[h][:, 64:65])
                nc.vector.tensor_scalar_mul(
                    hsq[:, h * 64:(h + 1) * 64], accs[h][:, 0:64], rec[:])
            tr = ps_pr.tile([128, SQW], F16, tag="pr", name=f"tr_{t}")
            nc.tensor.transpose(tr[:, 0:128], hsq[:], ident_sb[:])
            nc.vector.tensor_copy(
                headst_sb[:, t * 128:(t + 1) * 128], tr[:, 0:128])
            ot = outp.tile([128, D_M], F16, tag="ot", name=f"ot_{t}")
            for dmc in range(D_M // SQW):
                op = ps_pr.tile([128, SQW], F32, tag="pr", name=f"op_{t}_{dmc}")
                nc.tensor.matmul(
                    op[:],
                    headst_sb[:, t * 128:(t + 1) * 128],
                    wot_sb[:, dmc * SQW:(dmc + 1) * SQW],
                    start=True,
                    stop=True,
                )
                dst = ot[:, dmc * SQW:(dmc + 1) * SQW]
                if copy_split and dmc % 2:
                    # last group: ACT is idle after the exps -> share
                    nc.scalar.copy(dst, op[:])
                else:
                    nc.vector.tensor_copy(dst, op[:])
            nc.sync.dma_start(out[t * 128:(t + 1) * 128, :], ot[:])

        # ---- DMA stream (SP queue, in order) ----
        nc.sync.dma_start(ident_sb[:], ident)
        tk, tq, tv = {}, {}, {}
        tk[0] = load_chunk(kt, 0, "kc_0")
        load_w(wk, wk_sb)
        tq[0] = load_chunk(qt, 0, "qc_0")
        load_w(wq, wq_sb)
        load_w(wv, wv_sb)
        nc.sync.dma_start(wot_sb[:], wot)
        tk[1] = load_chunk(kt, 1, "kc_1")
        tq[1] = load_chunk(qt, 1, "qc_1")
        tv[0] = load_chunk(vt, 0, "vc_0")
        tk[2] = load_chunk(kt, 2, "kc_2")
        tq[2] = load_chunk(qt, 2, "qc_2")
        tv[1] = load_chunk(vt, 1, "vc_1")
        tk[3] = load_chunk(kt, 3, "kc_3")
        tq[3] = load_chunk(qt, 3, "qc_3")
        tv[2] = load_chunk(vt, 2, "vc_2")
        tv[3] = load_chunk(vt, 3, "vc_3")

        # ones column of VpAug via gpsimd memset (no DMA needed)
        nc.gpsimd.memset(
            vpa_sb[:].rearrange("p (c f) -> p c f", f=65)[:, :, 64:65], 1.0)

        # exp bias constant (-4) as a per-partition scalar AP
        bias_sb = wpool.tile([128, 1], F32, tag="bias")
        nc.gpsimd.memset(bias_sb[:], -4.0)

        # absorb the 1.3us exp table load inside the initial DMA window
        warm = smalls.tile([128, 1], F32, tag="warm")
        nc.scalar.activation(warm[:], ident_sb[:, 0:1], EXP, scale=1.0)

        # ---- compute stream ----
        # c=0: only K0 x Q0 feasible (4 ET tiles)
        project(tk[0], wk_sb, kpt_sb, 0, "pk0")
        project(tq[0], wq_sb, qpt_sb, 0, "pq0")
        for jj in (0, 1):
            z_exp(0, jj)
        # c=1
        project(tk[1], wk_sb, kpt_sb, 1, "pk1")
        project(tq[1], wq_sb, qpt_sb, 1, "pq1")
        for (g, jj) in ((0, 2), (0, 3), (1, 0), (1, 1), (1, 2), (1, 3)):
            z_exp(g, jj)
        project_v(tv[0], 0)
        # c=2
        project(tk[2], wk_sb, kpt_sb, 2, "pk2")
        project(tq[2], wq_sb, qpt_sb, 2, "pq2")
        for (g, jj) in ((0, 4), (0, 5), (1, 4), (1, 5),
                        (2, 0), (2, 1), (2, 2), (2, 3), (2, 4), (2, 5)):
            z_exp(g, jj)
        project_v(tv[1], 1)
        # c=3
        project(tk[3], wk_sb, kpt_sb, 3, "pk3")
        project(tq[3], wq_sb, qpt_sb, 3, "pq3")
        z_exp(0, 6)
        z_exp(0, 7)
        z_exp(1, 6)
        z_exp(1, 7)
        project_v(tv[2], 2)
        z_exp(2, 6)
        z_exp(2, 7)
        project_v(tv[3], 3)
        # group 3's z/exp stream with groups 0-2's finish units interleaved
        # (the units' AV runs become runnable as those groups' last exps land)
        units = [(g, m) for g in (0, 1, 2) for m in range(4)]
        ui = 0
        for jj in range(NJJ):
            z_exp(3, jj)
            for _ in range(2 if 2 <= jj < 6 else 1):
                if ui < len(units):
                    unit(*units[ui])
                    ui += 1
        while ui < len(units):
            unit(*units[ui])
            ui += 1
        for m in range(4):
            unit(3, m, copy_split=True)
def _build_nc():
    nc = bacc.Bacc("TRN2", target_bir_lowering=False, debug=False,
                   num_devices=NCORES)
    aps = {
        "qt": nc.dram_tensor("qt", [D_X, S], F16, kind="ExternalInput").ap(),
        "kt": nc.dram_tensor("kt", [D_X, S], F16, kind="ExternalInput").ap(),
        "vt": nc.dram_tensor("vt", [D_X, S], F16, kind="ExternalInput").ap(),
        "wq": nc.dram_tensor("wq", [D_X, HD], F16, kind="ExternalInput").ap(),
        "wk": nc.dram_tensor("wk", [D_X, HD], F16, kind="ExternalInput").ap(),
        "wv": nc.dram_tensor("wv", [D_X, HD], F16, kind="ExternalInput").ap(),
        "wot": nc.dram_tensor("wot", [HD, D_M], F16, kind="ExternalInput").ap(),
        "ident": nc.dram_tensor("ident", [128, 128], F16, kind="ExternalInput").ap(),
        "out": nc.dram_tensor("out", [S, D_M], F16, kind="ExternalOutput").ap(),
    }
    with tile.TileContext(nc) as tc:
        with nc.allow_low_precision(reason="fp16 matmul/softmax pipeline"):
            _emit(tc, nc, aps)
    nc.compile()
    return nc


def kernel(**inputs):
    global LAST_EXEC_NS, _NC_CACHE
    Q = np.asarray(inputs["Q"], dtype=np.float32)
    K = np.asarray(inputs["K"], dtype=np.float32)
    V = np.asarray(inputs["V"], dtype=np.float32)
    W_q = np.asarray(inputs["W_q"], dtype=np.float32)
    W_k = np.asarray(inputs["W_k"], dtype=np.float32)
    W_v = np.asarray(inputs["W_v"], dtype=np.float32)
    W_o = np.asarray(inputs["W_o"], dtype=np.float32)

    QT = np.ascontiguousarray(Q.T.astype(np.float16))
    KT = np.ascontiguousarray(K.T.astype(np.float16))
    VT = np.ascontiguousarray(V.T.astype(np.float16))
    ident = np.eye(128, dtype=np.float16)
    in_maps = []
    for c in range(NCORES):
        h0 = HPC * c
        in_maps.append({
            "qt": QT, "kt": KT, "vt": VT,
            "wq": np.ascontiguousarray(np.concatenate(
                [W_q[h0 + i] for i in range(HPC)], axis=1).astype(np.float16)),
            "wk": np.ascontiguousarray(np.concatenate(
                [W_k[h0 + i] for i in range(HPC)], axis=1).astype(np.float16)),
            "wv": np.ascontiguousarray(np.concatenate(
                [W_v[h0 + i] for i in range(HPC)], axis=1).astype(np.float16)),
            "wot": np.ascontiguousarray(
                W_o[:, c * HD:(c + 1) * HD].T.astype(np.float16)),
            "ident": ident,
        })

    if _NC_CACHE is None:
        _NC_CACHE = _build_nc()
    nc = _NC_CACHE

    trace = bool(os.environ.get("MHA_TRACE"))
    res = None
    if trace:
        try:
            res = run_bass_kernel_spmd(nc, in_maps, list(range(NCORES)),
                                       trace=True)
        except Exception as e:  # profiling infra unavailable -> run untraced
            print(f"[kernel] traced run failed ({e!r}); falling back")
            res = None
    if res is None:
        res = run_bass_kernel_spmd(nc, in_maps, list(range(NCORES)))

    LAST_EXEC_NS = getattr(res, "exec_time_ns", None)

    out = np.zeros((S, D_M), np.float32)
    for r in res.results:
        out += r["out"].astype(np.float32)
    return out


# revision 9
# speedup vs baseline: 1.1906x; 1.0552x over previous
"""Multi-head attention (16 heads, S=2048, d_model=1024, d_head=64) on 8 TRN2
NeuronCores, tensor-parallel over heads (2 heads per core).

Restructured from the 120us baseline around the TimelineSim cost model, where
matmul cost = output-free-size rows (K and M are free):

  * AV matmuls run transposed: out[sq=128, dv+1=65] with the exp tile as the
    stationary operand, 65 rows per accumulation step instead of 512 -> PE
    drops from 196k to 166k rows (~82us -> ~69us busy).
  * softmax normalize becomes a per-partition tensor_scalar (denominator is
    column 64 of the accumulator) -- no gpsimd broadcast.
  * z PSUM tiles are [128, 1024] (2 banks); exp runs 64x instead of 128x,
    halving the fixed per-instruction ACT overhead (~81us -> ~66us busy).
  * everything 2-byte: fp16 in/out of every matmul, exp computes
    exp(z/8 - 4) so the scores fit fp16 (max z/8 ~ 11.9), the bias cancels
    in the normalize. Output ships fp16 (half the out DMA), host sums in f32.
  * heads come out of AV as [sq, dv]; a 16x[128,128] PE transpose (+DVE copy)
    restores [hd, sq] for the output projection.

Schedule: inputs stream K0,Q0,K1,Q1,V0,K2,Q2,V1,K3,Q3,V2,V3 so the exp
stream (the ~66us ACT critical path, the pacer) starts by ~7us and never
starves. z/exp tiles are emitted greedily (group-ascending); group 0's AV
rides inline behind the V projections; groups 1-3 accumulate at the end of
the stream in group order, each followed by its normalize/transpose/outproj/
DMA chain so only group 3's chain is a tail. PSUM: 2x[128,1024] z +
2x[128,260] AV accumulators + 2x[128,512] proj/outproj = exactly 8 banks.
"""

import os

import numpy as np

import concourse.bass as bass
import concourse.tile as tile
from concourse import bacc, mybir
from concourse.bass_utils import run_bass_kernel_spmd

HEADS, D_K, D_V, D_X, D_M, S = 16, 64, 64, 1024, 1024, 2048
NCORES = 8
HPC = HEADS // NCORES          # heads per core
HD = HPC * D_K                 # 128: stacked head dim per core
SQW = 512                      # sq group width
NSQ = S // SQW                 # 4 groups
SKW = 128                      # sk chunk width (partition dim)
NSK = S // SKW                 # 16
NXC = D_X // 128               # 8 contraction chunks for projections
NJJ = NSK // 2                 # 8 skc-pairs (one [128,1024] z tile each)

F32 = mybir.dt.float32
F16 = mybir.dt.float16
EXP = mybir.ActivationFunctionType.Exp

LAST_EXEC_NS = None
_NC_CACHE = None


def _emit(tc, nc, aps):
    from contextlib import ExitStack

    qt, kt, vt, wq, wk, wv, wot, ident, out = (
        aps["qt"], aps["kt"], aps["vt"], aps["wq"], aps["wk"], aps["wv"],
        aps["wot"], aps["ident"], aps["out"],
    )

    with ExitStack() as ctx:
        wpool = ctx.enter_context(tc.tile_pool(name="weights", bufs=1))
        proj = ctx.enter_context(tc.tile_pool(name="proj", bufs=1))
        inp = ctx.enter_context(tc.tile_pool(name="inp", bufs=6))
        etp = ctx.enter_context(tc.tile_pool(name="et", bufs=52))
        hsqp = ctx.enter_context(tc.tile_pool(name="hsq", bufs=8))
        outp = ctx.enter_context(tc.tile_pool(name="outs", bufs=4))
        smalls = ctx.enter_context(tc.tile_pool(name="smalls", bufs=4))
        ps_z = ctx.enter_context(tc.tile_pool(name="ps_z", bufs=2, space="PSUM"))
        ps_av = ctx.enter_context(tc.tile_pool(name="ps_av", bufs=2, space="PSUM"))
        ps_pr = ctx.enter_context(tc.tile_pool(name="ps_pr", bufs=2, space="PSUM"))

        # ---- persistent SBUF tensors ----
        wq_sb = wpool.tile([128, D_X], F16, tag="wq")     # (xc p) stacked chunks
        wk_sb = wpool.tile([128, D_X], F16, tag="wk")
        wv_sb = wpool.tile([128, D_X], F16, tag="wv")
        wot_sb = wpool.tile([HD, D_M], F16, tag="wot")
        ident_sb = wpool.tile([128, 128], F16, tag="ident")
        qpt_sb = proj.tile([HD, S], F16, tag="qpt")
        kpt_sb = proj.tile([HD, S], F16, tag="kpt")
        # VpAug: per (h, skc) a (128 sk, 65) block: cols 0-63 = Vp, col 64 = 1
        vpa_sb = proj.tile([128, HPC * NSK * 65], F16, tag="vpa")
        headst_sb = proj.tile([HD, S], F16, tag="headst")

        def load_w(w_dram, w_sb):
            nc.sync.dma_start(w_sb[:], w_dram)

        def load_chunk(tt_dram, c, name):
            """One DMA: all 8 xc strips of a 512-wide chunk -> (128, 8, 512)."""
            t = inp.tile([128, NXC, SQW], F16, tag="inp", name=name)
            nc.sync.dma_start(
                t[:],
                tt_dram.rearrange("(xc p) s -> p xc s", p=128)[
                    :, :, c * SQW:(c + 1) * SQW
                ],
            )
            return t

        def project(t, w_sb, dst_sb, c, name):
            """dst_sb[:, c*512:(c+1)*512] = W.T @ X.T chunk (fp16)."""
            ps = ps_pr.tile([128, SQW], F32, tag="pr", name=name)
            for xc in range(NXC):
                nc.tensor.matmul(
                    ps[:],
                    w_sb[:, xc * 128:(xc + 1) * 128],
                    t[:, xc, :],
                    start=(xc == 0),
                    stop=(xc == NXC - 1),
                )
            nc.vector.tensor_copy(dst_sb[:, c * SQW:(c + 1) * SQW], ps[:])

        def project_v(t, c):
            """VpAug sk-chunks for 512-chunk c: Vp = VT_chunk.T @ Wv directly
            in (sk, hd) layout."""
            for j in range(SQW // SKW):
                skc = c * (SQW // SKW) + j
                ps = ps_pr.tile([128, HD], F32, tag="pr", name=f"vp_{skc}",
                                padded_shape=[128, SQW])
                for xc in range(NXC):
                    nc.tensor.matmul(
                        ps[:],
                        t[:, xc, j * SKW:(j + 1) * SKW],
                        wv_sb[:, xc * 128:(xc + 1) * 128],
                        start=(xc == 0),
                        stop=(xc == NXC - 1),
                    )
                for h in range(HPC):
                    base = (h * NSK + skc) * 65
                    nc.vector.tensor_copy(
                        vpa_sb[:, base:base + 64],
                        ps[:, h * 64:(h + 1) * 64],
                    )

        ets = {}  # (h, g, jj) -> ET tile awaiting its AV matmuls

        def z_exp(g, jj):
            """Per head: one [128,1024] z tile (skc pair 2jj,2jj+1) + exp.

            exp(z/8 - 4): the -4 bias keeps the scores in fp16 range
            (max z/8 ~ 11.9 -> e^7.9 = 2.7e3) and cancels in the normalize.
            """
            for h in range(HPC):
                z_ps = ps_z.tile([128, 2 * SQW], F32, tag="z",
                                 name=f"z_{h}_{g}_{jj}")
                for half in range(2):
                    skc = 2 * jj + half
                    nc.tensor.matmul(
                        z_ps[:, half * SQW:(half + 1) * SQW],
                        kpt_sb[h * 64:(h + 1) * 64, skc * SKW:(skc + 1) * SKW],
                        qpt_sb[h * 64:(h + 1) * 64, g * SQW:(g + 1) * SQW],
                        start=True,
                        stop=True,
                    )
                et = etp.tile([128, 2 * SQW], F16, tag="et",
                              name=f"et_{h}_{g}_{jj}")
                nc.scalar.activation(et[:], z_ps[:], EXP,
                                     scale=1.0 / 8.0, bias=bias_sb[:])
                ets[(h, g, jj)] = et

        # PSUM accumulation groups must be contiguous per bank (interleaved
        # start/stop groups at different offsets in one bank corrupt the
        # result), so AV runs as per-(h, sq-128-subtile) bursts of 16
        # back-to-back matmuls, one bank each, after the group's exps.
        def av_run(g, h, m):
            acc = ps_av.tile([128, 65], F32, tag="av", name=f"av_{g}_{h}_{m}")
            for jj in range(NJJ):
                et = ets[(h, g, jj)]
                for half in range(2):
                    skc = 2 * jj + half
                    vb = (h * NSK + skc) * 65
                    nc.tensor.matmul(
                        acc[:],
                        et[:, half * SQW + m * 128:half * SQW + (m + 1) * 128],
                        vpa_sb[:, vb:vb + 65],
                        start=(skc == 0),
                        stop=(skc == NSK - 1),
                    )
            return acc

        def unit(g, m, tail=False):
            """One sq-128 tile end to end: 2 AV runs -> normalize ->
            transpose -> output projection -> DMA.

            Mid-stream (ACT busy with exps) everything non-PE runs on DVE;
            in the tail (ACT idle) the muls/copies shift to ACT so the
            serial PE<->DVE chain shortens.
            """
            t = g * 4 + m
            accs = [av_run(g, h, m) for h in range(HPC)]
            hsq = hsqp.tile([128, HD], F16, tag="hsq", name=f"hsq_{t}")
            for h in range(HPC):
                rec = smalls.tile([128, 1], F32, tag="rec", name=f"rec_{t}_{h}")
                nc.vector.reciprocal(rec[:], accs[h][:, 64:65])
                dsth = hsq[:, h * 64:(h + 1) * 64]
                if tail:
                    nc.scalar.mul(dsth, accs[h][:, 0:64], rec[:])
                else:
                    nc.vector.tensor_scalar_mul(dsth, accs[h][:, 0:64], rec[:])
            tr = ps_pr.tile([128, SQW], F16, tag="pr", name=f"tr_{t}")
            nc.tensor.transpose(tr[:, 0:128], hsq[:], ident_sb[:])
            hdst = headst_sb[:, t * 128:(t + 1) * 128]
            if tail:
                nc.scalar.copy(hdst, tr[:, 0:128])
            else:
                nc.vector.tensor_copy(hdst, tr[:, 0:128])
            ot = outp.tile([128, D_M], F16, tag="ot", name=f"ot_{t}")
            for dmc in range(D_M // SQW):
                op = ps_pr.tile([128, SQW], F32, tag="pr", name=f"op_{t}_{dmc}")
                nc.tensor.matmul(
                    op[:],
                    headst_sb[:, t * 128:(t + 1) * 128],
                    wot_sb[:, dmc * SQW:(dmc + 1) * SQW],
                    start=True,
                    stop=True,
                )
                dst = ot[:, dmc * SQW:(dmc + 1) * SQW]
                if tail and dmc % 2:
                    nc.scalar.copy(dst, op[:])
                else:
                    nc.vector.tensor_copy(dst, op[:])
            nc.sync.dma_start(out[t * 128:(t + 1) * 128, :], ot[:])

        # ---- DMA stream (SP queue, in order) ----
        # weights first (now full-rate, ~0.7us each) so the K0/Q0 projections
        # start the exp stream as early as possible
        nc.sync.dma_start(ident_sb[:], ident)
        load_w(wk, wk_sb)
        load_w(wq, wq_sb)
        tk, tq, tv = {}, {}, {}
        tk[0] = load_chunk(kt, 0, "kc_0")
        tq[0] = load_chunk(qt, 0, "qc_0")
        tk[1] = load_chunk(kt, 1, "kc_1")
        tq[1] = load_chunk(qt, 1, "qc_1")
        load_w(wv, wv_sb)
        tv[0] = load_chunk(vt, 0, "vc_0")
        nc.sync.dma_start(wot_sb[:], wot)
        tk[2] = load_chunk(kt, 2, "kc_2")
        tq[2] = load_chunk(qt, 2, "qc_2")
        tv[1] = load_chunk(vt, 1, "vc_1")
        tk[3] = load_chunk(kt, 3, "kc_3")
        tq[3] = load_chunk(qt, 3, "qc_3")
        tv[2] = load_chunk(vt, 2, "vc_2")
        tv[3] = load_chunk(vt, 3, "vc_3")

        # ones column of VpAug via gpsimd memset (no DMA needed)
        nc.gpsimd.memset(
            vpa_sb[:].rearrange("p (c f) -> p c f", f=65)[:, :, 64:65], 1.0)

        # exp bias constant (-4) as a per-partition scalar AP
        bias_sb = wpool.tile([128, 1], F32, tag="bias")
        nc.gpsimd.memset(bias_sb[:], -4.0)

        # absorb the 1.3us exp table load inside the initial DMA window
        warm = smalls.tile([128, 1], F32, tag="warm")
        nc.scalar.activation(warm[:], ident_sb[:, 0:1], EXP, scale=1.0)

        # burn the PE pstate ramp (low/mid clock for the first ~3us of a busy
        # stretch) on junk matmuls during the input-DMA window, so the first
        # projections run at full clock
        junk = ps_pr.tile([128, SQW], F32, tag="pr", name="junk")
        for _ in range(36):
            nc.tensor.matmul(junk[:, 0:128], ident_sb[:], ident_sb[:],
                             start=True, stop=True)

        # ---- compute stream ----
        # c=0: only K0 x Q0 feasible (4 ET tiles)
        project(tk[0], wk_sb, kpt_sb, 0, "pk0")
        project(tq[0], wq_sb, qpt_sb, 0, "pq0")
        for jj in (0, 1):
            z_exp(0, jj)
        # c=1
        project(tk[1], wk_sb, kpt_sb, 1, "pk1")
        project(tq[1], wq_sb, qpt_sb, 1, "pq1")
        for (g, jj) in ((0, 2), (0, 3), (1, 0), (1, 1), (1, 2), (1, 3)):
            z_exp(g, jj)
        project_v(tv[0], 0)
        # c=2
        project(tk[2], wk_sb, kpt_sb, 2, "pk2")
        project(tq[2], wq_sb, qpt_sb, 2, "pq2")
        for (g, jj) in ((0, 4), (0, 5), (1, 4), (1, 5),
                        (2, 0), (2, 1), (2, 2), (2, 3), (2, 4), (2, 5)):
            z_exp(g, jj)
        project_v(tv[1], 1)
        # c=3
        project(tk[3], wk_sb, kpt_sb, 3, "pk3")
        project(tq[3], wq_sb, qpt_sb, 3, "pq3")
        z_exp(0, 6)
        z_exp(0, 7)
        z_exp(1, 6)
        z_exp(1, 7)
        project_v(tv[2], 2)
        z_exp(2, 6)
        z_exp(2, 7)
        project_v(tv[3], 3)
        # group 3's z/exp stream with groups 0-2's finish units interleaved
        # (the units' AV runs become runnable as those groups' last exps land)
        units = [(g, m) for g in (0, 1, 2) for m in range(4)]
        ui = 0
        for jj in range(NJJ):
            z_exp(3, jj)
            for _ in range(2 if 2 <= jj < 6 else 1):
                if ui < len(units):
                    unit(*units[ui])
                    ui += 1
        while ui < len(units):
            unit(*units[ui])
            ui += 1
        for m in range(4):
            unit(3, m, tail=True)
def _build_nc():
    nc = bacc.Bacc("TRN2", target_bir_lowering=False, debug=False,
                   num_devices=NCORES)
    aps = {
        "qt": nc.dram_tensor("qt", [D_X, S], F16, kind="ExternalInput").ap(),
        "kt": nc.dram_tensor("kt", [D_X, S], F16, kind="ExternalInput").ap(),
        "vt": nc.dram_tensor("vt", [D_X, S], F16, kind="ExternalInput").ap(),
        "wq": nc.dram_tensor("wq", [128, D_X], F16, kind="ExternalInput").ap(),
        "wk": nc.dram_tensor("wk", [128, D_X], F16, kind="ExternalInput").ap(),
        "wv": nc.dram_tensor("wv", [128, D_X], F16, kind="ExternalInput").ap(),
        "wot": nc.dram_tensor("wot", [HD, D_M], F16, kind="ExternalInput").ap(),
        "ident": nc.dram_tensor("ident", [128, 128], F16, kind="ExternalInput").ap(),
        "out": nc.dram_tensor("out", [S, D_M], F16, kind="ExternalOutput").ap(),
    }
    with tile.TileContext(nc) as tc:
        with nc.allow_low_precision(reason="fp16 matmul/softmax pipeline"):
            _emit(tc, nc, aps)
    nc.compile()
    return nc


def kernel(**inputs):
    global LAST_EXEC_NS, _NC_CACHE
    Q = np.asarray(inputs["Q"], dtype=np.float32)
    K = np.asarray(inputs["K"], dtype=np.float32)
    V = np.asarray(inputs["V"], dtype=np.float32)
    W_q = np.asarray(inputs["W_q"], dtype=np.float32)
    W_k = np.asarray(inputs["W_k"], dtype=np.float32)
    W_v = np.asarray(inputs["W_v"], dtype=np.float32)
    W_o = np.asarray(inputs["W_o"], dtype=np.float32)

    def _pack_w(W, h0):
        # device SBUF layout [p, c*128+f] = W[c*128+p, f]; full-rate DMA rows
        w = np.concatenate([W[h0 + i] for i in range(HPC)], axis=1)  # (D_X, HD)
        return np.ascontiguousarray(
            w.reshape(NXC, 128, HD).transpose(1, 0, 2).reshape(128, NXC * HD)
            .astype(np.float16))

    QT = np.ascontiguousarray(Q.T.astype(np.float16))
    KT = np.ascontiguousarray(K.T.astype(np.float16))
    VT = np.ascontiguousarray(V.T.astype(np.float16))
    ident = np.eye(128, dtype=np.float16)
    in_maps = []
    for c in range(NCORES):
        h0 = HPC * c
        in_maps.append({
            "qt": QT, "kt": KT, "vt": VT,
            "wq": _pack_w(W_q, h0), "wk": _pack_w(W_k, h0),
            "wv": _pack_w(W_v, h0),
            "wot": np.ascontiguousarray(
                W_o[:, c * HD:(c + 1) * HD].T.astype(np.float16)),
            "ident": ident,
        })

    if _NC_CACHE is None:
        _NC_CACHE = _build_nc()
    nc = _NC_CACHE

    trace = bool(os.environ.get("MHA_TRACE"))
    res = None
    if trace:
        try:
            res = run_bass_kernel_spmd(nc, in_maps, list(range(NCORES)),
                                       trace=True)
        except Exception as e:  # profiling infra unavailable -> run untraced
            print(f"[kernel] traced run failed ({e!r}); falling back")
            res = None
    if res is None:
        res = run_bass_kernel_spmd(nc, in_maps, list(range(NCORES)))

    LAST_EXEC_NS = getattr(res, "exec_time_ns", None)

    out = np.zeros((S, D_M), np.float32)
    for r in res.results:
        out += r["out"].astype(np.float32)
    return out
